# revision 2
# baseline (speedup 1.0000x reference)
"""Strassen-1 (fp16) + flat fp8-DoubleRow hybrid column-parallel linear.

out = input_ @ weight.T + bias, F-sharded 8 ways; per-core C[8192,2048].

The contraction K=4096 splits into K16 fp16 planes + K8 = 256*A8 fp8 planes.
 - fp16 part: one level of Strassen over (M, K16, F): 7 products, each
   [4096, K16/2] @ [K16/2, 1024], host precomputes operand combos.
   C11 = M1+M4-M5+M7; C12 = M3+M5; C21 = M2+M4; C22 = M1-M2+M3+M6.
 - fp8 part: plain e4m3 DoubleRow GEMM over K8 planes (no Strassen: the
   recombination would amplify fp8 error ~2x). Folded into the same psum
   banks: E11 -> M7 (single-use in C11), E22 -> M6 (single-use in C22);
   E12 / E21 get their own banks.
PE time ~= (0.875*(K16/K) + 0.5*(K8/K)) * fp16-roofline ~= 0.734 -> ~1.31 ms.
All W scaled by 64 (e4m3 normal range); copyback divides by 64 + bias.
"""

import os
import sys

import numpy as np
import ml_dtypes

for _p in ("/opt/trn_rl_repo", "/root/.axon_site/_ro/trn_rl_repo"):
    if os.path.isdir(_p) and _p not in sys.path:
        sys.path.insert(0, _p)

P = 128
S, B, H, F = 4096, 2, 4096, 16384
N_CORES = 8
M = S * B
FS = F // N_CORES
W_SCALE = 64.0

A8 = 6  # fp8 256-plane blocks (alpha = A8/16)
K8 = 256 * A8
K16 = H - K8          # 2560
KH = K16 // 2         # 1280 (Strassen half-K)
KTH = KH // P         # 10
MH = M // 2           # 4096
RT = MH // P          # 32 row tiles per half
FH = FS // 2          # 1024 (abstract half-F)


def build_nc():
    from concourse import bacc
    import concourse.mybir as mybir
    import concourse.tile as tile

    f32 = mybir.dt.float32
    fp16 = mybir.dt.float16
    fp8 = mybir.dt.float8e4
    DR = mybir.MatmulPerfMode.DoubleRow
    ALU = mybir.AluOpType

    nc = bacc.Bacc(None, target_bir_lowering=False)
    # at[p, r, k, kt, m] = fp16(Acombo_p[r*P + m, kt*P + k])
    at = nc.declare_dram_parameter("at", [7, RT, P, KTH, P], fp16, isOutput=False)
    # bt[p, k, h, kt, f] = fp16(64 * Bcombo_p[kt*P + k, h*512 + f])
    bt = nc.declare_dram_parameter("bt", [7, P, 2, KTH, 512], fp16, isOutput=False)
    # xt8[R, k, j, i, m] = fp8(x[R*P + m, K16 + j*256 + i*128 + k])
    xt8 = nc.declare_dram_parameter("xt8", [2 * RT, P, A8, 2, P], fp8, isOutput=False)
    # wt8[k, j, i, f] = fp8(64 * w[f, K16 + j*256 + i*128 + k])
    wt8 = nc.declare_dram_parameter("wt8", [P, A8, 2, FS], fp8, isOutput=False)
    bias = nc.declare_dram_parameter("bias", [P, FS], f32, isOutput=False)
    out = nc.declare_dram_parameter("out", [M, FS], f32, isOutput=True)

    with tile.TileContext(nc) as tc:
        with (
            tc.tile_pool(name="bpool7", bufs=7) as bpool7,
            tc.tile_pool(name="w8pool", bufs=A8) as w8pool,
            tc.tile_pool(name="apool", bufs=14) as apool,
            tc.tile_pool(name="x8pool", bufs=4) as x8pool,
            tc.tile_pool(name="tpool", bufs=14) as tpool,
            tc.tile_pool(name="opool", bufs=8) as opool,
            tc.tile_pool(name="biaspool", bufs=1) as biaspool,
            tc.tile_pool(name="psum", bufs=8, space="PSUM") as pspool,
        ):
            bias_sb = biaspool.tile([P, FS], f32)
            nc.scalar.dma_start(out=bias_sb[:, :], in_=bias[:, :])

            w8_kt = []
            for j in range(A8):
                wk8 = w8pool.tile([P, 2, FS], fp8, tag="w8kt")
                nc.scalar.dma_start(out=wk8[:, :, :], in_=wt8[:, j, :, :])
                w8_kt.append(wk8)

            def emit_dr(ps, x8t, ocol0, start):
                for q in (0, 1):
                    for j in range(A8):
                        nc.tensor.matmul(
                            ps[:, q * 256 : (q + 1) * 256],
                            lhsT=x8t[:, j, :, :],
                            rhs=w8_kt[j][:, :, ocol0 + q * 256 : ocol0 + q * 256 + 256],
                            start=(start and q == 0 and j == 0),
                            stop=(q == 1 and j == A8 - 1),
                            perf_mode=DR,
                        )

            for h in range(2):
                b_sb = []
                for p in range(7):
                    btile = bpool7.tile([P, KTH, 512], fp16, tag="btile")
                    nc.scalar.dma_start(out=btile[:, :, :], in_=bt[p, :, h, :, :])
                    b_sb.append(btile)
                cL = h * 512          # C11 / C21 out-col base
                cR = FH + h * 512     # C12 / C22 out-col base
                for r in range(RT):
                    a_sb = []
                    for p in range(7):
                        atile = apool.tile([P, KTH, P], fp16, tag="atile")
                        nc.sync.dma_start(out=atile[:, :, :], in_=at[p, r, :, :, :])
                        a_sb.append(atile)
                    x8_top = x8pool.tile([P, A8, 2, P], fp8, tag="x8")
                    nc.sync.dma_start(out=x8_top[:, :, :, :], in_=xt8[r, :, :, :, :])
                    x8_bot = x8pool.tile([P, A8, 2, P], fp8, tag="x8")
                    nc.sync.dma_start(
                        out=x8_bot[:, :, :, :], in_=xt8[RT + r, :, :, :, :]
                    )

                    def product(p_idx, fold=None):
                        ps = pspool.tile([P, 512], f32, tag="ps")
                        for kt in range(KTH):
                            nc.tensor.matmul(
                                ps[:, :],
                                lhsT=a_sb[p_idx][:, kt, :],
                                rhs=b_sb[p_idx][:, kt, :],
                                start=(kt == 0),
                                stop=(kt == KTH - 1 and fold is None),
                            )
                        if fold is not None:
                            emit_dr(ps, fold[0], fold[1], start=False)
                        return ps

                    # early-freed products first (bank reuse across the 9 tiles).
                    # DVE reads at most one PSUM operand per op, so m2/m4/m5 go
                    # through the (otherwise idle) scalar engine to SBUF first.
                    m1 = product(0)
                    m2 = product(1)
                    m4 = product(3)
                    m2_sb = tpool.tile([P, 512], f32, tag="t")
                    nc.scalar.copy(m2_sb[:, :], m2[:, :])
                    m4_sb = tpool.tile([P, 512], f32, tag="t")
                    nc.scalar.copy(m4_sb[:, :], m4[:, :])
                    t11 = tpool.tile([P, 512], f32, tag="t")
                    nc.vector.tensor_add(t11[:, :], m1[:, :], m4_sb[:, :])
                    t22 = tpool.tile([P, 512], f32, tag="t")
                    nc.vector.tensor_sub(t22[:, :], m1[:, :], m2_sb[:, :])
                    t21 = tpool.tile([P, 512], f32, tag="t")
                    nc.vector.tensor_add(t21[:, :], m2_sb[:, :], m4_sb[:, :])

                    m5 = product(4)
                    m5_sb = tpool.tile([P, 512], f32, tag="t")
                    nc.scalar.copy(m5_sb[:, :], m5[:, :])
                    m3 = product(2)
                    m7 = product(6, fold=(x8_top, cL))   # + E11
                    m6 = product(5, fold=(x8_bot, cR))   # + E22
                    e12 = pspool.tile([P, 512], f32, tag="ps")
                    emit_dr(e12, x8_top, cR, start=True)
                    e21 = pspool.tile([P, 512], f32, tag="ps")
                    emit_dr(e21, x8_bot, cL, start=True)

                    r_top = r * P
                    r_bot = MH + r * P

                    # C11 = t11 - M5 + M7'
                    u1 = tpool.tile([P, 512], f32, tag="t")
                    nc.vector.tensor_sub(u1[:, :], t11[:, :], m5_sb[:, :])
                    w1 = tpool.tile([P, 512], f32, tag="t")
                    nc.vector.tensor_add(w1[:, :], u1[:, :], m7[:, :])
                    o11 = opool.tile([P, 512], f32, tag="o")
                    nc.vector.scalar_tensor_tensor(
                        out=o11[:, :], in0=w1[:, :], scalar=1.0 / W_SCALE,
                        in1=bias_sb[:, cL : cL + 512],
                        op0=ALU.mult, op1=ALU.add,
                    )
                    nc.scalar.dma_start(
                        out=out[r_top : r_top + P, cL : cL + 512], in_=o11[:, :]
                    )
                    # C12 = M3 + M5 + E12
                    u3 = tpool.tile([P, 512], f32, tag="t")
                    nc.vector.tensor_add(u3[:, :], m3[:, :], m5_sb[:, :])
                    w3 = tpool.tile([P, 512], f32, tag="t")
                    nc.vector.tensor_add(w3[:, :], u3[:, :], e12[:, :])
                    o12 = opool.tile([P, 512], f32, tag="o")
                    nc.vector.scalar_tensor_tensor(
                        out=o12[:, :], in0=w3[:, :], scalar=1.0 / W_SCALE,
                        in1=bias_sb[:, cR : cR + 512],
                        op0=ALU.mult, op1=ALU.add,
                    )
                    nc.scalar.dma_start(
                        out=out[r_top : r_top + P, cR : cR + 512], in_=o12[:, :]
                    )
                    # C21 = t21 + E21
                    w4 = tpool.tile([P, 512], f32, tag="t")
                    nc.vector.tensor_add(w4[:, :], t21[:, :], e21[:, :])
                    o21 = opool.tile([P, 512], f32, tag="o")
                    nc.vector.scalar_tensor_tensor(
                        out=o21[:, :], in0=w4[:, :], scalar=1.0 / W_SCALE,
                        in1=bias_sb[:, cL : cL + 512],
                        op0=ALU.mult, op1=ALU.add,
                    )
                    nc.scalar.dma_start(
                        out=out[r_bot : r_bot + P, cL : cL + 512], in_=o21[:, :]
                    )
                    # C22 = t22 + M3 + M6'
                    u2 = tpool.tile([P, 512], f32, tag="t")
                    nc.vector.tensor_add(u2[:, :], t22[:, :], m3[:, :])
                    w2 = tpool.tile([P, 512], f32, tag="t")
                    nc.vector.tensor_add(w2[:, :], u2[:, :], m6[:, :])
                    o22 = opool.tile([P, 512], f32, tag="o")
                    nc.vector.scalar_tensor_tensor(
                        out=o22[:, :], in0=w2[:, :], scalar=1.0 / W_SCALE,
                        in1=bias_sb[:, cR : cR + 512],
                        op0=ALU.mult, op1=ALU.add,
                    )
                    nc.scalar.dma_start(
                        out=out[r_bot : r_bot + P, cR : cR + 512], in_=o22[:, :]
                    )
    nc.compile()
    return nc


def make_in_maps(input_, weight, bias):
    e4m3 = ml_dtypes.float8_e4m3
    X = np.asarray(input_, dtype=np.float32).reshape(M, H)
    X16 = X[:, :K16]
    A11 = X16[:MH, :KH]
    A12 = X16[:MH, KH:]
    A21 = X16[MH:, :KH]
    A22 = X16[MH:, KH:]
    acombos = [A11 + A22, A21 + A22, A11, A22, A11 + A12, A21 - A11, A12 - A22]
    # at[p, r, k, kt, m] = Acombo_p[r*P+m, kt*P+k]
    at = np.stack(
        [
            a.astype(np.float16).reshape(RT, P, KTH, P).transpose(0, 3, 2, 1)
            for a in acombos
        ]
    )
    at = np.ascontiguousarray(at)
    X8 = X[:, K16:].astype(e4m3)
    xt8 = np.ascontiguousarray(X8.reshape(2 * RT, P, A8, 2, P).transpose(0, 4, 2, 3, 1))
    b = np.asarray(bias, dtype=np.float32)
    Wall = np.asarray(weight, dtype=np.float32) * W_SCALE
    in_maps = []
    for c in range(N_CORES):
        Wc = Wall[c * FS : (c + 1) * FS]  # [FS, H]
        Bm = Wc[:, :K16].T  # [K16, FS]
        B11 = Bm[:KH, :FH]
        B12 = Bm[:KH, FH:]
        B21 = Bm[KH:, :FH]
        B22 = Bm[KH:, FH:]
        bcombos = [B11 + B22, B11, B12 - B22, B21 - B11, B22, B11 + B12, B21 + B22]
        # bt[p, k, h, kt, f] = Bcombo_p[kt*P+k, h*512+f]
        btc = np.stack(
            [
                bm.astype(np.float16).reshape(KTH, P, 2, 512).transpose(1, 2, 0, 3)
                for bm in bcombos
            ]
        )
        btc = np.ascontiguousarray(btc)
        W8c = Wc[:, K16:].astype(e4m3)  # [FS, K8]
        wt8c = np.ascontiguousarray(W8c.reshape(FS, A8, 2, P).transpose(3, 1, 2, 0))
        bc = np.ascontiguousarray(
            np.broadcast_to(b[c * FS : (c + 1) * FS][None, :], (P, FS))
        )
        in_maps.append({"at": at, "bt": btc, "xt8": xt8, "wt8": wt8c, "bias": bc})
    return in_maps


_NC_CACHE = {}


def run_spmd(input_, weight, bias, trace=False, **kw):
    from concourse.bass_utils import run_bass_kernel_spmd

    if "full" not in _NC_CACHE:
        _NC_CACHE["full"] = build_nc()
    nc = _NC_CACHE["full"]
    in_maps = make_in_maps(input_, weight, bias)
    res = run_bass_kernel_spmd(
        nc, in_maps, core_ids=list(range(N_CORES)), trace=trace, **kw
    )
    outs = [np.asarray(res.results[c]["out"]) for c in range(N_CORES)]
    full = np.concatenate(outs, axis=1).reshape(S, B, F)
    return full, res


def kernel(input_, weight, bias):
    out, _ = run_spmd(input_, weight, bias, trace=False)
    return out


# revision 3
# speedup vs baseline: 1.0014x; 1.0014x over previous
"""Strassen-1 (fp16) + flat fp8-DoubleRow hybrid column-parallel linear.

out = input_ @ weight.T + bias, F-sharded 8 ways; per-core C[8192,2048].

The contraction K=4096 splits into K16 fp16 planes + K8 = 256*A8 fp8 planes.
 - fp16 part: one level of Strassen over (M, K16, F): 7 products, each
   [4096, K16/2] @ [K16/2, 1024], host precomputes operand combos.
   C11 = M1+M4-M5+M7; C12 = M3+M5; C21 = M2+M4; C22 = M1-M2+M3+M6.
 - fp8 part: plain e4m3 DoubleRow GEMM over K8 planes (no Strassen: the
   recombination would amplify fp8 error ~2x). Folded into the same psum
   banks: E11 -> M7 (single-use in C11), E22 -> M6 (single-use in C22);
   E12 / E21 get their own banks.
PE time ~= (0.875*(K16/K) + 0.5*(K8/K)) * fp16-roofline ~= 0.734 -> ~1.31 ms.
All W scaled by 64 (e4m3 normal range); copyback divides by 64 + bias.
"""

import os
import sys

import numpy as np
import ml_dtypes

for _p in ("/opt/trn_rl_repo", "/root/.axon_site/_ro/trn_rl_repo"):
    if os.path.isdir(_p) and _p not in sys.path:
        sys.path.insert(0, _p)

P = 128
S, B, H, F = 4096, 2, 4096, 16384
N_CORES = 8
M = S * B
FS = F // N_CORES
W_SCALE = 64.0

A8 = 6  # fp8 256-plane blocks (alpha = A8/16)
K8 = 256 * A8
K16 = H - K8          # 2560
KH = K16 // 2         # 1280 (Strassen half-K)
KTH = KH // P         # 10
MH = M // 2           # 4096
RT = MH // P          # 32 row tiles per half
FH = FS // 2          # 1024 (abstract half-F)


def build_nc():
    from concourse import bacc
    import concourse.mybir as mybir
    import concourse.tile as tile

    f32 = mybir.dt.float32
    fp16 = mybir.dt.float16
    fp8 = mybir.dt.float8e4
    DR = mybir.MatmulPerfMode.DoubleRow
    ALU = mybir.AluOpType

    nc = bacc.Bacc(None, target_bir_lowering=False)
    # at[p, r, k, kt, m] = fp16(Acombo_p[r*P + m, kt*P + k])
    at = nc.declare_dram_parameter("at", [7, RT, P, KTH, P], fp16, isOutput=False)
    # bt[p, k, h, kt, f] = fp16(64 * Bcombo_p[kt*P + k, h*512 + f])
    bt = nc.declare_dram_parameter("bt", [7, P, 2, KTH, 512], fp16, isOutput=False)
    # xt8[R, k, j, i, m] = fp8(x[R*P + m, K16 + j*256 + i*128 + k])
    xt8 = nc.declare_dram_parameter("xt8", [2 * RT, P, A8, 2, P], fp8, isOutput=False)
    # wt8[k, j, i, f] = fp8(64 * w[f, K16 + j*256 + i*128 + k])
    wt8 = nc.declare_dram_parameter("wt8", [P, A8, 2, FS], fp8, isOutput=False)
    bias = nc.declare_dram_parameter("bias", [P, FS], f32, isOutput=False)
    out = nc.declare_dram_parameter("out", [M, FS], f32, isOutput=True)

    with tile.TileContext(nc) as tc:
        with (
            tc.tile_pool(name="bpool7", bufs=8) as bpool7,
            tc.tile_pool(name="w8pool", bufs=A8) as w8pool,
            tc.tile_pool(name="apool", bufs=14) as apool,
            tc.tile_pool(name="x8pool", bufs=4) as x8pool,
            tc.tile_pool(name="tpool", bufs=14) as tpool,
            tc.tile_pool(name="opool", bufs=8) as opool,
            tc.tile_pool(name="biaspool", bufs=1) as biaspool,
            tc.tile_pool(name="psum", bufs=8, space="PSUM") as pspool,
        ):
            # emitted before the h-loop, but AFTER the first B tiles below so the
            # scalar DMA queue delivers the first products' operands first
            bias_sb = biaspool.tile([P, FS], f32)
            w8_kt = []
            for j in range(A8):
                wk8 = w8pool.tile([P, 2, FS], fp8, tag="w8kt")
                w8_kt.append(wk8)
            _startup_emitted = [False]

            def emit_startup_loads():
                for j in range(A8):
                    nc.scalar.dma_start(out=w8_kt[j][:, :, :], in_=wt8[:, j, :, :])
                nc.scalar.dma_start(out=bias_sb[:, :], in_=bias[:, :])

            def emit_dr(ps, x8t, ocol0, start):
                # j outer: consecutive matmuls share the same stationary lhsT
                for j in range(A8):
                    for q in (0, 1):
                        nc.tensor.matmul(
                            ps[:, q * 256 : (q + 1) * 256],
                            lhsT=x8t[:, j, :, :],
                            rhs=w8_kt[j][:, :, ocol0 + q * 256 : ocol0 + q * 256 + 256],
                            start=(start and q == 0 and j == 0),
                            stop=(q == 1 and j == A8 - 1),
                            perf_mode=DR,
                        )

            for h in range(2):
                b_sb = []
                for p in range(7):
                    btile = bpool7.tile([P, KTH, 512], fp16, tag="btile")
                    # kt-granular chunks: the first matmuls of this half wait on
                    # 250KB, not the full 8.75MB B reload (h-boundary stall)
                    for k0 in range(0, KTH, 2):
                        nc.scalar.dma_start(
                            out=btile[:, k0 : k0 + 2, :],
                            in_=bt[p, :, h, k0 : k0 + 2, :],
                        )
                    b_sb.append(btile)
                if not _startup_emitted[0]:
                    _startup_emitted[0] = True
                    emit_startup_loads()
                cL = h * 512          # C11 / C21 out-col base
                cR = FH + h * 512     # C12 / C22 out-col base
                for r in range(RT):
                    a_sb = []
                    for p in range(7):
                        atile = apool.tile([P, KTH, P], fp16, tag="atile")
                        nc.sync.dma_start(out=atile[:, :, :], in_=at[p, r, :, :, :])
                        a_sb.append(atile)
                    x8_top = x8pool.tile([P, A8, 2, P], fp8, tag="x8")
                    nc.sync.dma_start(out=x8_top[:, :, :, :], in_=xt8[r, :, :, :, :])
                    x8_bot = x8pool.tile([P, A8, 2, P], fp8, tag="x8")
                    nc.sync.dma_start(
                        out=x8_bot[:, :, :, :], in_=xt8[RT + r, :, :, :, :]
                    )

                    def product(p_idx, fold=None):
                        ps = pspool.tile([P, 512], f32, tag="ps")
                        for kt in range(KTH):
                            nc.tensor.matmul(
                                ps[:, :],
                                lhsT=a_sb[p_idx][:, kt, :],
                                rhs=b_sb[p_idx][:, kt, :],
                                start=(kt == 0),
                                stop=(kt == KTH - 1 and fold is None),
                            )
                        if fold is not None:
                            emit_dr(ps, fold[0], fold[1], start=False)
                        return ps

                    # early-freed products first (bank reuse across the 9 tiles).
                    # DVE reads at most one PSUM operand per op, so m2/m4/m5 go
                    # through the (otherwise idle) scalar engine to SBUF first.
                    m1 = product(0)
                    m2 = product(1)
                    m4 = product(3)
                    m2_sb = tpool.tile([P, 512], f32, tag="t")
                    nc.scalar.copy(m2_sb[:, :], m2[:, :])
                    m4_sb = tpool.tile([P, 512], f32, tag="t")
                    nc.scalar.copy(m4_sb[:, :], m4[:, :])
                    t11 = tpool.tile([P, 512], f32, tag="t")
                    nc.vector.tensor_add(t11[:, :], m1[:, :], m4_sb[:, :])
                    t22 = tpool.tile([P, 512], f32, tag="t")
                    nc.vector.tensor_sub(t22[:, :], m1[:, :], m2_sb[:, :])
                    t21 = tpool.tile([P, 512], f32, tag="t")
                    nc.vector.tensor_add(t21[:, :], m2_sb[:, :], m4_sb[:, :])

                    m5 = product(4)
                    m5_sb = tpool.tile([P, 512], f32, tag="t")
                    nc.scalar.copy(m5_sb[:, :], m5[:, :])
                    m3 = product(2)
                    m7 = product(6, fold=(x8_top, cL))   # + E11
                    m6 = product(5, fold=(x8_bot, cR))   # + E22
                    e12 = pspool.tile([P, 512], f32, tag="ps")
                    emit_dr(e12, x8_top, cR, start=True)
                    e21 = pspool.tile([P, 512], f32, tag="ps")
                    emit_dr(e21, x8_bot, cL, start=True)

                    r_top = r * P
                    r_bot = MH + r * P

                    # C11 = t11 - M5 + M7'
                    u1 = tpool.tile([P, 512], f32, tag="t")
                    nc.vector.tensor_sub(u1[:, :], t11[:, :], m5_sb[:, :])
                    w1 = tpool.tile([P, 512], f32, tag="t")
                    nc.vector.tensor_add(w1[:, :], u1[:, :], m7[:, :])
                    o11 = opool.tile([P, 512], f32, tag="o")
                    nc.vector.scalar_tensor_tensor(
                        out=o11[:, :], in0=w1[:, :], scalar=1.0 / W_SCALE,
                        in1=bias_sb[:, cL : cL + 512],
                        op0=ALU.mult, op1=ALU.add,
                    )
                    nc.scalar.dma_start(
                        out=out[r_top : r_top + P, cL : cL + 512], in_=o11[:, :]
                    )
                    # C12 = M3 + M5 + E12
                    u3 = tpool.tile([P, 512], f32, tag="t")
                    nc.vector.tensor_add(u3[:, :], m3[:, :], m5_sb[:, :])
                    w3 = tpool.tile([P, 512], f32, tag="t")
                    nc.vector.tensor_add(w3[:, :], u3[:, :], e12[:, :])
                    o12 = opool.tile([P, 512], f32, tag="o")
                    nc.vector.scalar_tensor_tensor(
                        out=o12[:, :], in0=w3[:, :], scalar=1.0 / W_SCALE,
                        in1=bias_sb[:, cR : cR + 512],
                        op0=ALU.mult, op1=ALU.add,
                    )
                    nc.scalar.dma_start(
                        out=out[r_top : r_top + P, cR : cR + 512], in_=o12[:, :]
                    )
                    # C21 = t21 + E21
                    w4 = tpool.tile([P, 512], f32, tag="t")
                    nc.vector.tensor_add(w4[:, :], t21[:, :], e21[:, :])
                    o21 = opool.tile([P, 512], f32, tag="o")
                    nc.vector.scalar_tensor_tensor(
                        out=o21[:, :], in0=w4[:, :], scalar=1.0 / W_SCALE,
                        in1=bias_sb[:, cL : cL + 512],
                        op0=ALU.mult, op1=ALU.add,
                    )
                    nc.scalar.dma_start(
                        out=out[r_bot : r_bot + P, cL : cL + 512], in_=o21[:, :]
                    )
                    # C22 = t22 + M3 + M6'
                    u2 = tpool.tile([P, 512], f32, tag="t")
                    nc.vector.tensor_add(u2[:, :], t22[:, :], m3[:, :])
                    w2 = tpool.tile([P, 512], f32, tag="t")
                    nc.vector.tensor_add(w2[:, :], u2[:, :], m6[:, :])
                    o22 = opool.tile([P, 512], f32, tag="o")
                    nc.vector.scalar_tensor_tensor(
                        out=o22[:, :], in0=w2[:, :], scalar=1.0 / W_SCALE,
                        in1=bias_sb[:, cR : cR + 512],
                        op0=ALU.mult, op1=ALU.add,
                    )
                    nc.scalar.dma_start(
                        out=out[r_bot : r_bot + P, cR : cR + 512], in_=o22[:, :]
                    )
    nc.compile()
    return nc


def make_in_maps(input_, weight, bias):
    e4m3 = ml_dtypes.float8_e4m3
    X = np.asarray(input_, dtype=np.float32).reshape(M, H)
    X16 = X[:, :K16]
    A11 = X16[:MH, :KH]
    A12 = X16[:MH, KH:]
    A21 = X16[MH:, :KH]
    A22 = X16[MH:, KH:]
    acombos = [A11 + A22, A21 + A22, A11, A22, A11 + A12, A21 - A11, A12 - A22]
    # at[p, r, k, kt, m] = Acombo_p[r*P+m, kt*P+k]
    at = np.stack(
        [
            a.astype(np.float16).reshape(RT, P, KTH, P).transpose(0, 3, 2, 1)
            for a in acombos
        ]
    )
    at = np.ascontiguousarray(at)
    X8 = X[:, K16:].astype(e4m3)
    xt8 = np.ascontiguousarray(X8.reshape(2 * RT, P, A8, 2, P).transpose(0, 4, 2, 3, 1))
    b = np.asarray(bias, dtype=np.float32)
    Wall = np.asarray(weight, dtype=np.float32) * W_SCALE
    in_maps = []
    for c in range(N_CORES):
        Wc = Wall[c * FS : (c + 1) * FS]  # [FS, H]
        Bm = Wc[:, :K16].T  # [K16, FS]
        B11 = Bm[:KH, :FH]
        B12 = Bm[:KH, FH:]
        B21 = Bm[KH:, :FH]
        B22 = Bm[KH:, FH:]
        bcombos = [B11 + B22, B11, B12 - B22, B21 - B11, B22, B11 + B12, B21 + B22]
        # bt[p, k, h, kt, f] = Bcombo_p[kt*P+k, h*512+f]
        btc = np.stack(
            [
                bm.astype(np.float16).reshape(KTH, P, 2, 512).transpose(1, 2, 0, 3)
                for bm in bcombos
            ]
        )
        btc = np.ascontiguousarray(btc)
        W8c = Wc[:, K16:].astype(e4m3)  # [FS, K8]
        wt8c = np.ascontiguousarray(W8c.reshape(FS, A8, 2, P).transpose(3, 1, 2, 0))
        bc = np.ascontiguousarray(
            np.broadcast_to(b[c * FS : (c + 1) * FS][None, :], (P, FS))
        )
        in_maps.append({"at": at, "bt": btc, "xt8": xt8, "wt8": wt8c, "bias": bc})
    return in_maps


_NC_CACHE = {}


def run_spmd(input_, weight, bias, trace=False, **kw):
    from concourse.bass_utils import run_bass_kernel_spmd

    if "full" not in _NC_CACHE:
        _NC_CACHE["full"] = build_nc()
    nc = _NC_CACHE["full"]
    in_maps = make_in_maps(input_, weight, bias)
    res = run_bass_kernel_spmd(
        nc, in_maps, core_ids=list(range(N_CORES)), trace=trace, **kw
    )
    outs = [np.asarray(res.results[c]["out"]) for c in range(N_CORES)]
    full = np.concatenate(outs, axis=1).reshape(S, B, F)
    return full, res


def kernel(input_, weight, bias):
    out, _ = run_spmd(input_, weight, bias, trace=False)
    return out


# revision 4
# speedup vs baseline: 1.0118x; 1.0104x over previous
"""Strassen-1 (fp16) + flat fp8-DoubleRow hybrid column-parallel linear.

out = input_ @ weight.T + bias, F-sharded 8 ways; per-core C[8192,2048].

The contraction K=4096 splits into K16 fp16 planes + K8 = 256*A8 fp8 planes.
 - fp16 part: one level of Strassen over (M, K16, F): 7 products, each
   [4096, K16/2] @ [K16/2, 1024], host precomputes operand combos.
   C11 = M1+M4-M5+M7; C12 = M3+M5; C21 = M2+M4; C22 = M1-M2+M3+M6.
 - fp8 part: plain e4m3 DoubleRow GEMM over K8 planes (no Strassen: the
   recombination would amplify fp8 error ~2x). Folded into the same psum
   banks: E11 -> M7 (single-use in C11), E22 -> M6 (single-use in C22);
   E12 / E21 get their own banks.
PE time ~= (0.875*(K16/K) + 0.5*(K8/K)) * fp16-roofline ~= 0.734 -> ~1.31 ms.
All W scaled by 64 (e4m3 normal range); copyback divides by 64 + bias.

Measured: HW exec 1375806 ns (baseline fp16 kernel: 1801760 ns, 1.31x),
rel err 1.9485e-2 (gate 2e-2; fp8 part contributes 3.18e-2 * sqrt(6/16),
bit-stable across runs). fp8 DoubleRow on TRN2 is 2x fp16 rate, so pure-fp8
(3.18e-2) and hi/lo-corrected fp8 schemes cannot pass the gate any faster;
Strassen-2 is DMA/DVE-bound. This sits within ~4% of the structural floor
of the Strassen-1 + alpha-split family.
"""

import os
import sys

import numpy as np
import ml_dtypes

for _p in ("/opt/trn_rl_repo", "/root/.axon_site/_ro/trn_rl_repo"):
    if os.path.isdir(_p) and _p not in sys.path:
        sys.path.insert(0, _p)

P = 128
S, B, H, F = 4096, 2, 4096, 16384
N_CORES = 8
M = S * B
FS = F // N_CORES
W_SCALE = 64.0

A8 = 6  # fp8 256-plane blocks (alpha = A8/16)
K8 = 256 * A8
K16 = H - K8          # 2560
KH = K16 // 2         # 1280 (Strassen half-K)
KTH = KH // P         # 10
MH = M // 2           # 4096
RT = MH // P          # 32 row tiles per half
FH = FS // 2          # 1024 (abstract half-F)


def build_nc():
    from concourse import bacc
    import concourse.mybir as mybir
    import concourse.tile as tile

    f32 = mybir.dt.float32
    fp16 = mybir.dt.float16
    fp8 = mybir.dt.float8e4
    DR = mybir.MatmulPerfMode.DoubleRow
    ALU = mybir.AluOpType

    nc = bacc.Bacc(None, target_bir_lowering=False)
    # at[p, r, k, kt, m] = fp16(Acombo_p[r*P + m, kt*P + k])
    at = nc.declare_dram_parameter("at", [7, RT, P, KTH, P], fp16, isOutput=False)
    # bt[p, k, h, kt, f] = fp16(64 * Bcombo_p[kt*P + k, h*512 + f])
    bt = nc.declare_dram_parameter("bt", [7, P, 2, KTH, 512], fp16, isOutput=False)
    # xt8[R, k, j, i, m] = fp8(x[R*P + m, K16 + j*256 + i*128 + k])
    xt8 = nc.declare_dram_parameter("xt8", [2 * RT, P, A8, 2, P], fp8, isOutput=False)
    # wt8[k, j, i, f] = fp8(64 * w[f, K16 + j*256 + i*128 + k])
    wt8 = nc.declare_dram_parameter("wt8", [P, A8, 2, FS], fp8, isOutput=False)
    bias = nc.declare_dram_parameter("bias", [P, FS], f32, isOutput=False)
    out = nc.declare_dram_parameter("out", [M, FS], f32, isOutput=True)

    with tile.TileContext(nc) as tc:
        with (
            tc.tile_pool(name="bpool7", bufs=8) as bpool7,
            tc.tile_pool(name="w8pool", bufs=A8) as w8pool,
            tc.tile_pool(name="apool", bufs=14) as apool,
            tc.tile_pool(name="x8pool", bufs=4) as x8pool,
            tc.tile_pool(name="tpool", bufs=14) as tpool,
            tc.tile_pool(name="opool", bufs=8) as opool,
            tc.tile_pool(name="biaspool", bufs=1) as biaspool,
            tc.tile_pool(name="psum", bufs=8, space="PSUM") as pspool,
        ):
            # emitted before the h-loop, but AFTER the first B tiles below so the
            # scalar DMA queue delivers the first products' operands first
            bias_sb = biaspool.tile([P, FS], f32)
            w8_kt = []
            for j in range(A8):
                wk8 = w8pool.tile([P, 2, FS], fp8, tag="w8kt")
                w8_kt.append(wk8)
            _startup_emitted = [False]

            def emit_startup_loads():
                for j in range(A8):
                    nc.scalar.dma_start(out=w8_kt[j][:, :, :], in_=wt8[:, j, :, :])
                nc.scalar.dma_start(out=bias_sb[:, :], in_=bias[:, :])

            def emit_dr(ps, x8t, ocol0, start):
                # j outer: consecutive matmuls share the same stationary lhsT
                for j in range(A8):
                    for q in (0, 1):
                        nc.tensor.matmul(
                            ps[:, q * 256 : (q + 1) * 256],
                            lhsT=x8t[:, j, :, :],
                            rhs=w8_kt[j][:, :, ocol0 + q * 256 : ocol0 + q * 256 + 256],
                            start=(start and q == 0 and j == 0),
                            stop=(q == 1 and j == A8 - 1),
                            perf_mode=DR,
                        )

            for h in range(2):
                b_sb = []
                for p in range(7):
                    btile = bpool7.tile([P, KTH, 512], fp16, tag="btile")
                    # kt-granular chunks: the first matmuls of this half wait on
                    # 250KB, not the full 8.75MB B reload (h-boundary stall)
                    for k0 in range(0, KTH, 2):
                        nc.scalar.dma_start(
                            out=btile[:, k0 : k0 + 2, :],
                            in_=bt[p, :, h, k0 : k0 + 2, :],
                        )
                    b_sb.append(btile)
                if not _startup_emitted[0]:
                    _startup_emitted[0] = True
                    emit_startup_loads()
                cL = h * 512          # C11 / C21 out-col base
                cR = FH + h * 512     # C12 / C22 out-col base
                for r in range(RT):
                    a_sb = []
                    for p in range(7):
                        atile = apool.tile([P, KTH, P], fp16, tag="atile")
                        nc.sync.dma_start(out=atile[:, :, :], in_=at[p, r, :, :, :])
                        a_sb.append(atile)
                    x8_top = x8pool.tile([P, A8, 2, P], fp8, tag="x8")
                    nc.sync.dma_start(out=x8_top[:, :, :, :], in_=xt8[r, :, :, :, :])
                    x8_bot = x8pool.tile([P, A8, 2, P], fp8, tag="x8")
                    nc.sync.dma_start(
                        out=x8_bot[:, :, :, :], in_=xt8[RT + r, :, :, :, :]
                    )

                    def product(p_idx, fold=None):
                        ps = pspool.tile([P, 512], f32, tag="ps")
                        for kt in range(KTH):
                            nc.tensor.matmul(
                                ps[:, :],
                                lhsT=a_sb[p_idx][:, kt, :],
                                rhs=b_sb[p_idx][:, kt, :],
                                start=(kt == 0),
                                stop=(kt == KTH - 1 and fold is None),
                            )
                        if fold is not None:
                            emit_dr(ps, fold[0], fold[1], start=False)
                        return ps

                    # early-freed products first (bank reuse across the 9 tiles).
                    # DVE reads at most one PSUM operand per op, so m2/m4/m5 go
                    # through the (otherwise idle) scalar engine to SBUF first.
                    m1 = product(0)
                    m2 = product(1)
                    m4 = product(3)
                    m2_sb = tpool.tile([P, 512], f32, tag="t")
                    nc.scalar.copy(m2_sb[:, :], m2[:, :])
                    m4_sb = tpool.tile([P, 512], f32, tag="t")
                    nc.scalar.copy(m4_sb[:, :], m4[:, :])
                    t11 = tpool.tile([P, 512], f32, tag="t")
                    nc.vector.tensor_add(t11[:, :], m1[:, :], m4_sb[:, :])
                    t22 = tpool.tile([P, 512], f32, tag="t")
                    nc.vector.tensor_sub(t22[:, :], m1[:, :], m2_sb[:, :])
                    t21 = tpool.tile([P, 512], f32, tag="t")
                    nc.vector.tensor_add(t21[:, :], m2_sb[:, :], m4_sb[:, :])

                    m5 = product(4)
                    m5_sb = tpool.tile([P, 512], f32, tag="t")
                    nc.scalar.copy(m5_sb[:, :], m5[:, :])
                    m3 = product(2)
                    m7 = product(6, fold=(x8_top, cL))   # + E11
                    m6 = product(5, fold=(x8_bot, cR))   # + E22
                    e12 = pspool.tile([P, 512], f32, tag="ps")
                    emit_dr(e12, x8_top, cR, start=True)
                    e21 = pspool.tile([P, 512], f32, tag="ps")
                    emit_dr(e21, x8_bot, cL, start=True)

                    r_top = r * P
                    r_bot = MH + r * P

                    # C11 = t11 - M5 + M7'
                    u1 = tpool.tile([P, 512], f32, tag="t")
                    nc.vector.tensor_sub(u1[:, :], t11[:, :], m5_sb[:, :])
                    w1 = tpool.tile([P, 512], f32, tag="t")
                    nc.vector.tensor_add(w1[:, :], u1[:, :], m7[:, :])
                    o11 = opool.tile([P, 512], f32, tag="o")
                    nc.vector.scalar_tensor_tensor(
                        out=o11[:, :], in0=w1[:, :], scalar=1.0 / W_SCALE,
                        in1=bias_sb[:, cL : cL + 512],
                        op0=ALU.mult, op1=ALU.add,
                    )
                    nc.scalar.dma_start(
                        out=out[r_top : r_top + P, cL : cL + 512], in_=o11[:, :]
                    )
                    # C12 = M3 + M5 + E12
                    u3 = tpool.tile([P, 512], f32, tag="t")
                    nc.vector.tensor_add(u3[:, :], m3[:, :], m5_sb[:, :])
                    w3 = tpool.tile([P, 512], f32, tag="t")
                    nc.vector.tensor_add(w3[:, :], u3[:, :], e12[:, :])
                    o12 = opool.tile([P, 512], f32, tag="o")
                    nc.vector.scalar_tensor_tensor(
                        out=o12[:, :], in0=w3[:, :], scalar=1.0 / W_SCALE,
                        in1=bias_sb[:, cR : cR + 512],
                        op0=ALU.mult, op1=ALU.add,
                    )
                    nc.scalar.dma_start(
                        out=out[r_top : r_top + P, cR : cR + 512], in_=o12[:, :]
                    )
                    # C21 = t21 + E21
                    w4 = tpool.tile([P, 512], f32, tag="t")
                    nc.vector.tensor_add(w4[:, :], t21[:, :], e21[:, :])
                    o21 = opool.tile([P, 512], f32, tag="o")
                    nc.vector.scalar_tensor_tensor(
                        out=o21[:, :], in0=w4[:, :], scalar=1.0 / W_SCALE,
                        in1=bias_sb[:, cL : cL + 512],
                        op0=ALU.mult, op1=ALU.add,
                    )
                    nc.scalar.dma_start(
                        out=out[r_bot : r_bot + P, cL : cL + 512], in_=o21[:, :]
                    )
                    # C22 = t22 + M3 + M6'
                    u2 = tpool.tile([P, 512], f32, tag="t")
                    nc.vector.tensor_add(u2[:, :], t22[:, :], m3[:, :])
                    w2 = tpool.tile([P, 512], f32, tag="t")
                    nc.vector.tensor_add(w2[:, :], u2[:, :], m6[:, :])
                    o22 = opool.tile([P, 512], f32, tag="o")
                    nc.vector.scalar_tensor_tensor(
                        out=o22[:, :], in0=w2[:, :], scalar=1.0 / W_SCALE,
                        in1=bias_sb[:, cR : cR + 512],
                        op0=ALU.mult, op1=ALU.add,
                    )
                    nc.scalar.dma_start(
                        out=out[r_bot : r_bot + P, cR : cR + 512], in_=o22[:, :]
                    )
    nc.compile()
    return nc


def make_in_maps(input_, weight, bias):
    e4m3 = ml_dtypes.float8_e4m3
    X = np.asarray(input_, dtype=np.float32).reshape(M, H)
    X16 = X[:, :K16]
    A11 = X16[:MH, :KH]
    A12 = X16[:MH, KH:]
    A21 = X16[MH:, :KH]
    A22 = X16[MH:, KH:]
    acombos = [A11 + A22, A21 + A22, A11, A22, A11 + A12, A21 - A11, A12 - A22]
    # at[p, r, k, kt, m] = Acombo_p[r*P+m, kt*P+k]
    at = np.stack(
        [
            a.astype(np.float16).reshape(RT, P, KTH, P).transpose(0, 3, 2, 1)
            for a in acombos
        ]
    )
    at = np.ascontiguousarray(at)
    X8 = X[:, K16:].astype(e4m3)
    xt8 = np.ascontiguousarray(X8.reshape(2 * RT, P, A8, 2, P).transpose(0, 4, 2, 3, 1))
    b = np.asarray(bias, dtype=np.float32)
    Wall = np.asarray(weight, dtype=np.float32) * W_SCALE
    in_maps = []
    for c in range(N_CORES):
        Wc = Wall[c * FS : (c + 1) * FS]  # [FS, H]
        Bm = Wc[:, :K16].T  # [K16, FS]
        B11 = Bm[:KH, :FH]
        B12 = Bm[:KH, FH:]
        B21 = Bm[KH:, :FH]
        B22 = Bm[KH:, FH:]
        bcombos = [B11 + B22, B11, B12 - B22, B21 - B11, B22, B11 + B12, B21 + B22]
        # bt[p, k, h, kt, f] = Bcombo_p[kt*P+k, h*512+f]
        btc = np.stack(
            [
                bm.astype(np.float16).reshape(KTH, P, 2, 512).transpose(1, 2, 0, 3)
                for bm in bcombos
            ]
        )
        btc = np.ascontiguousarray(btc)
        W8c = Wc[:, K16:].astype(e4m3)  # [FS, K8]
        wt8c = np.ascontiguousarray(W8c.reshape(FS, A8, 2, P).transpose(3, 1, 2, 0))
        bc = np.ascontiguousarray(
            np.broadcast_to(b[c * FS : (c + 1) * FS][None, :], (P, FS))
        )
        in_maps.append({"at": at, "bt": btc, "xt8": xt8, "wt8": wt8c, "bias": bc})
    return in_maps


_NC_CACHE = {}


def run_spmd(input_, weight, bias, trace=False, **kw):
    from concourse.bass_utils import run_bass_kernel_spmd

    if "full" not in _NC_CACHE:
        _NC_CACHE["full"] = build_nc()
    nc = _NC_CACHE["full"]
    in_maps = make_in_maps(input_, weight, bias)
    res = run_bass_kernel_spmd(
        nc, in_maps, core_ids=list(range(N_CORES)), trace=trace, **kw
    )
    outs = [np.asarray(res.results[c]["out"]) for c in range(N_CORES)]
    full = np.concatenate(outs, axis=1).reshape(S, B, F)
    return full, res


def kernel(input_, weight, bias):
    out, _ = run_spmd(input_, weight, bias, trace=False)
    return out


# revision 5
# speedup vs baseline: 1.0175x; 1.0057x over previous
"""Strassen-1 (fp16) + flat fp8-DoubleRow hybrid column-parallel linear.

out = input_ @ weight.T + bias, F-sharded 8 ways; per-core C[8192,2048].

The contraction K=4096 splits into K16 fp16 planes + K8 = 256*A8 fp8 planes.
 - fp16 part: one level of Strassen over (M, K16, F): 7 products, each
   [4096, K16/2] @ [K16/2, 1024], host precomputes operand combos.
   C11 = M1+M4-M5+M7; C12 = M3+M5; C21 = M2+M4; C22 = M1-M2+M3+M6.
 - fp8 part: plain e4m3 DoubleRow GEMM over K8 planes (no Strassen: the
   recombination would amplify fp8 error ~2x). Folded into the same psum
   banks: E11 -> M7 (single-use in C11), E22 -> M6 (single-use in C22);
   E12 / E21 get their own banks.
PE time ~= (0.875*(K16/K) + 0.5*(K8/K)) * fp16-roofline ~= 0.734 -> ~1.31 ms.
All W scaled by 64 (e4m3 normal range); copyback divides by 64 + bias.
"""

import os
import sys

import numpy as np
import ml_dtypes

for _p in ("/opt/trn_rl_repo", "/root/.axon_site/_ro/trn_rl_repo"):
    if os.path.isdir(_p) and _p not in sys.path:
        sys.path.insert(0, _p)

P = 128
S, B, H, F = 4096, 2, 4096, 16384
N_CORES = 8
M = S * B
FS = F // N_CORES
W_SCALE = 64.0

A8 = 6  # fp8 256-plane blocks (alpha = A8/16)
K8 = 256 * A8
K16 = H - K8          # 2560
KH = K16 // 2         # 1280 (Strassen half-K)
KTH = KH // P         # 10
MH = M // 2           # 4096
RT = MH // P          # 32 row tiles per half
FH = FS // 2          # 1024 (abstract half-F)


def build_nc():
    from concourse import bacc
    import concourse.mybir as mybir
    import concourse.tile as tile

    f32 = mybir.dt.float32
    fp16 = mybir.dt.float16
    fp8 = mybir.dt.float8e4
    DR = mybir.MatmulPerfMode.DoubleRow
    ALU = mybir.AluOpType

    nc = bacc.Bacc(None, target_bir_lowering=False)
    # at[p, r, k, kt, m] = fp16(Acombo_p[r*P + m, kt*P + k])
    at = nc.declare_dram_parameter("at", [7, RT, P, KTH, P], fp16, isOutput=False)
    # bt[p, k, h, kt, f] = fp16(64 * Bcombo_p[kt*P + k, h*512 + f])
    bt = nc.declare_dram_parameter("bt", [7, P, 2, KTH, 512], fp16, isOutput=False)
    # xt8[R, k, j, i, m] = fp8(x[R*P + m, K16 + j*256 + i*128 + k])
    xt8 = nc.declare_dram_parameter("xt8", [2 * RT, P, A8, 2, P], fp8, isOutput=False)
    # wt8[k, j, i, f] = fp8(64 * w[f, K16 + j*256 + i*128 + k])
    wt8 = nc.declare_dram_parameter("wt8", [P, A8, 2, FS], fp8, isOutput=False)
    bias = nc.declare_dram_parameter("bias", [P, FS], f32, isOutput=False)
    out = nc.declare_dram_parameter("out", [M, FS], f32, isOutput=True)

    with tile.TileContext(nc) as tc:
        with (
            tc.tile_pool(name="bpool7", bufs=8) as bpool7,
            tc.tile_pool(name="w8pool", bufs=A8) as w8pool,
            tc.tile_pool(name="apool", bufs=14) as apool,
            tc.tile_pool(name="x8pool", bufs=4) as x8pool,
            tc.tile_pool(name="tpool", bufs=14) as tpool,
            tc.tile_pool(name="opool", bufs=8) as opool,
            tc.tile_pool(name="biaspool", bufs=1) as biaspool,
            tc.tile_pool(name="psum", bufs=8, space="PSUM") as pspool,
        ):
            # w8/bias and two of the seven h=0 B tiles ride the sync queue,
            # emitted inside (h=0, r=0) after that iteration's A/x8 loads, so
            # both DMA rings deliver iteration 0's working set in parallel
            # (one scalar ring alone can't keep up with 7 products' B demand)
            bias_sb = biaspool.tile([P, FS], f32)
            w8_kt = []
            for j in range(A8):
                wk8 = w8pool.tile([P, 2, FS], fp8, tag="w8kt")
                w8_kt.append(wk8)

            def emit_dr(ps, x8t, ocol0, start):
                # j outer: consecutive matmuls share the same stationary lhsT
                for j in range(A8):
                    for q in (0, 1):
                        nc.tensor.matmul(
                            ps[:, q * 256 : (q + 1) * 256],
                            lhsT=x8t[:, j, :, :],
                            rhs=w8_kt[j][:, :, ocol0 + q * 256 : ocol0 + q * 256 + 256],
                            start=(start and q == 0 and j == 0),
                            stop=(q == 1 and j == A8 - 1),
                            perf_mode=DR,
                        )

            for h in range(2):
                b_sb = []
                # h=0: defer b2/b5 to the sync queue inside r=0 (products M3/M6
                # need them latest); scalar delivers b0,b1,b3,b4,b6 in need order
                deferred = (2, 5) if h == 0 else ()
                scalar_order = [p for p in (0, 1, 3, 4, 6, 2, 5) if p not in deferred]
                btiles = {}
                for p in range(7):
                    btile = bpool7.tile([P, KTH, 512], fp16, tag="btile")
                    btiles[p] = btile
                    b_sb.append(btile)
                for p in scalar_order:
                    # kt-granular chunks: the first matmuls of this half wait on
                    # 250KB, not the full 8.75MB B reload (h-boundary stall)
                    for k0 in range(0, KTH, 2):
                        nc.scalar.dma_start(
                            out=btiles[p][:, k0 : k0 + 2, :],
                            in_=bt[p, :, h, k0 : k0 + 2, :],
                        )
                cL = h * 512          # C11 / C21 out-col base
                cR = FH + h * 512     # C12 / C22 out-col base
                for r in range(RT):
                    a_sb = []
                    for p in range(7):
                        atile = apool.tile([P, KTH, P], fp16, tag="atile")
                        nc.sync.dma_start(out=atile[:, :, :], in_=at[p, r, :, :, :])
                        a_sb.append(atile)
                    x8_top = x8pool.tile([P, A8, 2, P], fp8, tag="x8")
                    nc.sync.dma_start(out=x8_top[:, :, :, :], in_=xt8[r, :, :, :, :])
                    x8_bot = x8pool.tile([P, A8, 2, P], fp8, tag="x8")
                    nc.sync.dma_start(
                        out=x8_bot[:, :, :, :], in_=xt8[RT + r, :, :, :, :]
                    )
                    if h == 0 and r == 0:
                        # startup extras on the sync ring, in need order: b2
                        # (M3, ~11us), b5 (M6, ~15us), w8 (folds, ~15us),
                        # bias (first copyback, ~20us)
                        for p in deferred:
                            for k0 in range(0, KTH, 2):
                                nc.sync.dma_start(
                                    out=btiles[p][:, k0 : k0 + 2, :],
                                    in_=bt[p, :, 0, k0 : k0 + 2, :],
                                )
                        for j in range(A8):
                            nc.sync.dma_start(
                                out=w8_kt[j][:, :, :], in_=wt8[:, j, :, :]
                            )
                        nc.sync.dma_start(out=bias_sb[:, :], in_=bias[:, :])

                    def product(p_idx, fold=None):
                        ps = pspool.tile([P, 512], f32, tag="ps")
                        for kt in range(KTH):
                            nc.tensor.matmul(
                                ps[:, :],
                                lhsT=a_sb[p_idx][:, kt, :],
                                rhs=b_sb[p_idx][:, kt, :],
                                start=(kt == 0),
                                stop=(kt == KTH - 1 and fold is None),
                            )
                        if fold is not None:
                            emit_dr(ps, fold[0], fold[1], start=False)
                        return ps

                    # early-freed products first (bank reuse across the 9 tiles).
                    # DVE reads at most one PSUM operand per op, so m2/m4/m5 go
                    # through the (otherwise idle) scalar engine to SBUF first.
                    m1 = product(0)
                    m2 = product(1)
                    m4 = product(3)
                    m2_sb = tpool.tile([P, 512], f32, tag="t")
                    nc.scalar.copy(m2_sb[:, :], m2[:, :])
                    m4_sb = tpool.tile([P, 512], f32, tag="t")
                    nc.scalar.copy(m4_sb[:, :], m4[:, :])
                    t11 = tpool.tile([P, 512], f32, tag="t")
                    nc.vector.tensor_add(t11[:, :], m1[:, :], m4_sb[:, :])
                    t22 = tpool.tile([P, 512], f32, tag="t")
                    nc.vector.tensor_sub(t22[:, :], m1[:, :], m2_sb[:, :])
                    t21 = tpool.tile([P, 512], f32, tag="t")
                    nc.vector.tensor_add(t21[:, :], m2_sb[:, :], m4_sb[:, :])

                    m5 = product(4)
                    m5_sb = tpool.tile([P, 512], f32, tag="t")
                    nc.scalar.copy(m5_sb[:, :], m5[:, :])
                    m3 = product(2)
                    # E12/E21 before the M7/M6 folds: their DVE chains (C21 is
                    # the shortest) start while M7/M6 matmuls still run, and the
                    # kernel tail isn't gated on e21 being the last psum
                    e12 = pspool.tile([P, 512], f32, tag="ps")
                    emit_dr(e12, x8_top, cR, start=True)
                    e21 = pspool.tile([P, 512], f32, tag="ps")
                    emit_dr(e21, x8_bot, cL, start=True)
                    m7 = product(6, fold=(x8_top, cL))   # + E11
                    m6 = product(5, fold=(x8_bot, cR))   # + E22

                    r_top = r * P
                    r_bot = MH + r * P

                    # C11 = t11 - M5 + M7'
                    u1 = tpool.tile([P, 512], f32, tag="t")
                    nc.vector.tensor_sub(u1[:, :], t11[:, :], m5_sb[:, :])
                    w1 = tpool.tile([P, 512], f32, tag="t")
                    nc.vector.tensor_add(w1[:, :], u1[:, :], m7[:, :])
                    o11 = opool.tile([P, 512], f32, tag="o")
                    nc.vector.scalar_tensor_tensor(
                        out=o11[:, :], in0=w1[:, :], scalar=1.0 / W_SCALE,
                        in1=bias_sb[:, cL : cL + 512],
                        op0=ALU.mult, op1=ALU.add,
                    )
                    nc.scalar.dma_start(
                        out=out[r_top : r_top + P, cL : cL + 512], in_=o11[:, :]
                    )
                    # C12 = M3 + M5 + E12
                    u3 = tpool.tile([P, 512], f32, tag="t")
                    nc.vector.tensor_add(u3[:, :], m3[:, :], m5_sb[:, :])
                    w3 = tpool.tile([P, 512], f32, tag="t")
                    nc.vector.tensor_add(w3[:, :], u3[:, :], e12[:, :])
                    o12 = opool.tile([P, 512], f32, tag="o")
                    nc.vector.scalar_tensor_tensor(
                        out=o12[:, :], in0=w3[:, :], scalar=1.0 / W_SCALE,
                        in1=bias_sb[:, cR : cR + 512],
                        op0=ALU.mult, op1=ALU.add,
                    )
                    nc.scalar.dma_start(
                        out=out[r_top : r_top + P, cR : cR + 512], in_=o12[:, :]
                    )
                    # C21 = t21 + E21
                    w4 = tpool.tile([P, 512], f32, tag="t")
                    nc.vector.tensor_add(w4[:, :], t21[:, :], e21[:, :])
                    o21 = opool.tile([P, 512], f32, tag="o")
                    nc.vector.scalar_tensor_tensor(
                        out=o21[:, :], in0=w4[:, :], scalar=1.0 / W_SCALE,
                        in1=bias_sb[:, cL : cL + 512],
                        op0=ALU.mult, op1=ALU.add,
                    )
                    nc.scalar.dma_start(
                        out=out[r_bot : r_bot + P, cL : cL + 512], in_=o21[:, :]
                    )
                    # C22 = t22 + M3 + M6'
                    u2 = tpool.tile([P, 512], f32, tag="t")
                    nc.vector.tensor_add(u2[:, :], t22[:, :], m3[:, :])
                    w2 = tpool.tile([P, 512], f32, tag="t")
                    nc.vector.tensor_add(w2[:, :], u2[:, :], m6[:, :])
                    o22 = opool.tile([P, 512], f32, tag="o")
                    nc.vector.scalar_tensor_tensor(
                        out=o22[:, :], in0=w2[:, :], scalar=1.0 / W_SCALE,
                        in1=bias_sb[:, cR : cR + 512],
                        op0=ALU.mult, op1=ALU.add,
                    )
                    nc.scalar.dma_start(
                        out=out[r_bot : r_bot + P, cR : cR + 512], in_=o22[:, :]
                    )
    nc.compile()
    return nc


def make_in_maps(input_, weight, bias):
    e4m3 = ml_dtypes.float8_e4m3
    X = np.asarray(input_, dtype=np.float32).reshape(M, H)
    X16 = X[:, :K16]
    A11 = X16[:MH, :KH]
    A12 = X16[:MH, KH:]
    A21 = X16[MH:, :KH]
    A22 = X16[MH:, KH:]
    acombos = [A11 + A22, A21 + A22, A11, A22, A11 + A12, A21 - A11, A12 - A22]
    # at[p, r, k, kt, m] = Acombo_p[r*P+m, kt*P+k]
    at = np.stack(
        [
            a.astype(np.float16).reshape(RT, P, KTH, P).transpose(0, 3, 2, 1)
            for a in acombos
        ]
    )
    at = np.ascontiguousarray(at)
    X8 = X[:, K16:].astype(e4m3)
    xt8 = np.ascontiguousarray(X8.reshape(2 * RT, P, A8, 2, P).transpose(0, 4, 2, 3, 1))
    b = np.asarray(bias, dtype=np.float32)
    Wall = np.asarray(weight, dtype=np.float32) * W_SCALE
    in_maps = []
    for c in range(N_CORES):
        Wc = Wall[c * FS : (c + 1) * FS]  # [FS, H]
        Bm = Wc[:, :K16].T  # [K16, FS]
        B11 = Bm[:KH, :FH]
        B12 = Bm[:KH, FH:]
        B21 = Bm[KH:, :FH]
        B22 = Bm[KH:, FH:]
        bcombos = [B11 + B22, B11, B12 - B22, B21 - B11, B22, B11 + B12, B21 + B22]
        # bt[p, k, h, kt, f] = Bcombo_p[kt*P+k, h*512+f]
        btc = np.stack(
            [
                bm.astype(np.float16).reshape(KTH, P, 2, 512).transpose(1, 2, 0, 3)
                for bm in bcombos
            ]
        )
        btc = np.ascontiguousarray(btc)
        W8c = Wc[:, K16:].astype(e4m3)  # [FS, K8]
        wt8c = np.ascontiguousarray(W8c.reshape(FS, A8, 2, P).transpose(3, 1, 2, 0))
        bc = np.ascontiguousarray(
            np.broadcast_to(b[c * FS : (c + 1) * FS][None, :], (P, FS))
        )
        in_maps.append({"at": at, "bt": btc, "xt8": xt8, "wt8": wt8c, "bias": bc})
    return in_maps


_NC_CACHE = {}


def run_spmd(input_, weight, bias, trace=False, **kw):
    from concourse.bass_utils import run_bass_kernel_spmd

    if "full" not in _NC_CACHE:
        _NC_CACHE["full"] = build_nc()
    nc = _NC_CACHE["full"]
    in_maps = make_in_maps(input_, weight, bias)
    res = run_bass_kernel_spmd(
        nc, in_maps, core_ids=list(range(N_CORES)), trace=trace, **kw
    )
    outs = [np.asarray(res.results[c]["out"]) for c in range(N_CORES)]
    full = np.concatenate(outs, axis=1).reshape(S, B, F)
    return full, res


def kernel(input_, weight, bias):
    out, _ = run_spmd(input_, weight, bias, trace=False)
    return out


# revision 6
# speedup vs baseline: 1.0195x; 1.0020x over previous
"""Strassen-1 (fp16) + flat fp8-DoubleRow hybrid column-parallel linear.

out = input_ @ weight.T + bias, F-sharded 8 ways; per-core C[8192,2048].

The contraction K=4096 splits into K16 fp16 planes + K8 = 256*A8 fp8 planes.
 - fp16 part: one level of Strassen over (M, K16, F): 7 products, each
   [4096, K16/2] @ [K16/2, 1024], host precomputes operand combos.
   C11 = M1+M4-M5+M7; C12 = M3+M5; C21 = M2+M4; C22 = M1-M2+M3+M6.
 - fp8 part: plain e4m3 DoubleRow GEMM over K8 planes (no Strassen: the
   recombination would amplify fp8 error ~2x). Folded into the same psum
   banks: E11 -> M7 (single-use in C11), E22 -> M6 (single-use in C22);
   E12 / E21 get their own banks.
PE time ~= (0.875*(K16/K) + 0.5*(K8/K)) * fp16-roofline ~= 0.734 -> ~1.31 ms.
All W scaled by 64 (e4m3 normal range); copyback divides by 64 + bias.
"""

import os
import sys

import numpy as np
import ml_dtypes

for _p in ("/opt/trn_rl_repo", "/root/.axon_site/_ro/trn_rl_repo"):
    if os.path.isdir(_p) and _p not in sys.path:
        sys.path.insert(0, _p)

P = 128
S, B, H, F = 4096, 2, 4096, 16384
N_CORES = 8
M = S * B
FS = F // N_CORES
W_SCALE = 64.0

A8 = 6  # fp8 256-plane blocks (alpha = A8/16)
K8 = 256 * A8
K16 = H - K8          # 2560
KH = K16 // 2         # 1280 (Strassen half-K)
KTH = KH // P         # 10
MH = M // 2           # 4096
RT = MH // P          # 32 row tiles per half
FH = FS // 2          # 1024 (abstract half-F)


def build_nc():
    from concourse import bacc
    import concourse.mybir as mybir
    import concourse.tile as tile

    f32 = mybir.dt.float32
    fp16 = mybir.dt.float16
    fp8 = mybir.dt.float8e4
    DR = mybir.MatmulPerfMode.DoubleRow
    ALU = mybir.AluOpType

    nc = bacc.Bacc(None, target_bir_lowering=False)
    # at[p, r, k, kt, m] = fp16(Acombo_p[r*P + m, kt*P + k])
    at = nc.declare_dram_parameter("at", [7, RT, P, KTH, P], fp16, isOutput=False)
    # bt[p, k, h, kt, f] = fp16(64 * Bcombo_p[kt*P + k, h*512 + f])
    bt = nc.declare_dram_parameter("bt", [7, P, 2, KTH, 512], fp16, isOutput=False)
    # xt8[R, k, j, i, m] = fp8(x[R*P + m, K16 + j*256 + i*128 + k])
    xt8 = nc.declare_dram_parameter("xt8", [2 * RT, P, A8, 2, P], fp8, isOutput=False)
    # wt8[k, j, i, f] = fp8(64 * w[f, K16 + j*256 + i*128 + k])
    wt8 = nc.declare_dram_parameter("wt8", [P, A8, 2, FS], fp8, isOutput=False)
    bias = nc.declare_dram_parameter("bias", [P, FS], f32, isOutput=False)
    out = nc.declare_dram_parameter("out", [M, FS], f32, isOutput=True)

    with tile.TileContext(nc) as tc:
        with (
            tc.tile_pool(name="bpool7", bufs=8) as bpool7,
            tc.tile_pool(name="w8pool", bufs=A8) as w8pool,
            tc.tile_pool(name="apool", bufs=14) as apool,
            tc.tile_pool(name="x8pool", bufs=4) as x8pool,
            tc.tile_pool(name="tpool", bufs=14) as tpool,
            tc.tile_pool(name="opool", bufs=8) as opool,
            tc.tile_pool(name="biaspool", bufs=1) as biaspool,
            tc.tile_pool(name="psum", bufs=8, space="PSUM") as pspool,
        ):
            # w8/bias and two of the seven h=0 B tiles ride the sync queue,
            # emitted inside (h=0, r=0) after that iteration's A/x8 loads, so
            # both DMA rings deliver iteration 0's working set in parallel
            # (one scalar ring alone can't keep up with 7 products' B demand)
            bias_sb = biaspool.tile([P, FS], f32)
            w8_kt = []
            for j in range(A8):
                wk8 = w8pool.tile([P, 2, FS], fp8, tag="w8kt")
                w8_kt.append(wk8)

            def emit_dr(ps, x8t, ocol0, start):
                # j outer: consecutive matmuls share the same stationary lhsT
                for j in range(A8):
                    for q in (0, 1):
                        nc.tensor.matmul(
                            ps[:, q * 256 : (q + 1) * 256],
                            lhsT=x8t[:, j, :, :],
                            rhs=w8_kt[j][:, :, ocol0 + q * 256 : ocol0 + q * 256 + 256],
                            start=(start and q == 0 and j == 0),
                            stop=(q == 1 and j == A8 - 1),
                            perf_mode=DR,
                        )

            for h in range(2):
                b_sb = []
                # defer b2/b5 to the sync queue inside r=0 of each half
                # (products M3/M6 need them latest); scalar delivers
                # b0,b1,b3,b4,b6 in need order
                deferred = (2, 5)
                scalar_order = [p for p in (0, 1, 3, 4, 6) if p not in deferred]
                btiles = {}
                for p in range(7):
                    btile = bpool7.tile([P, KTH, 512], fp16, tag="btile")
                    btiles[p] = btile
                    b_sb.append(btile)
                for p in scalar_order:
                    # kt-granular chunks: the first matmuls of this half wait on
                    # 250KB, not the full 8.75MB B reload (h-boundary stall)
                    for k0 in range(0, KTH, 2):
                        nc.scalar.dma_start(
                            out=btiles[p][:, k0 : k0 + 2, :],
                            in_=bt[p, :, h, k0 : k0 + 2, :],
                        )
                cL = h * 512          # C11 / C21 out-col base
                cR = FH + h * 512     # C12 / C22 out-col base
                for r in range(RT):
                    a_sb = []
                    for p in range(7):
                        atile = apool.tile([P, KTH, P], fp16, tag="atile")
                        nc.sync.dma_start(out=atile[:, :, :], in_=at[p, r, :, :, :])
                        a_sb.append(atile)
                    x8_top = x8pool.tile([P, A8, 2, P], fp8, tag="x8")
                    nc.sync.dma_start(out=x8_top[:, :, :, :], in_=xt8[r, :, :, :, :])
                    x8_bot = x8pool.tile([P, A8, 2, P], fp8, tag="x8")
                    nc.sync.dma_start(
                        out=x8_bot[:, :, :, :], in_=xt8[RT + r, :, :, :, :]
                    )
                    if r == 0:
                        # startup extras on the sync ring in need order: w8
                        # first (E12 folds hit it ~13us in), then b2 (M3,
                        # ~11us), b5 (M6, last product), bias (~20us)
                        if h == 0:
                            for j in range(A8):
                                nc.sync.dma_start(
                                    out=w8_kt[j][:, :, :], in_=wt8[:, j, :, :]
                                )
                        for p in deferred:
                            for k0 in range(0, KTH, 2):
                                nc.sync.dma_start(
                                    out=btiles[p][:, k0 : k0 + 2, :],
                                    in_=bt[p, :, h, k0 : k0 + 2, :],
                                )
                        if h == 0:
                            nc.sync.dma_start(out=bias_sb[:, :], in_=bias[:, :])

                    def product(p_idx, fold=None):
                        ps = pspool.tile([P, 512], f32, tag="ps")
                        for kt in range(KTH):
                            nc.tensor.matmul(
                                ps[:, :],
                                lhsT=a_sb[p_idx][:, kt, :],
                                rhs=b_sb[p_idx][:, kt, :],
                                start=(kt == 0),
                                stop=(kt == KTH - 1 and fold is None),
                            )
                        if fold is not None:
                            emit_dr(ps, fold[0], fold[1], start=False)
                        return ps

                    # early-freed products first (bank reuse across the 9 tiles).
                    # DVE reads at most one PSUM operand per op, so m2/m4/m5 go
                    # through the (otherwise idle) scalar engine to SBUF first.
                    m1 = product(0)
                    m2 = product(1)
                    m4 = product(3)
                    m2_sb = tpool.tile([P, 512], f32, tag="t")
                    nc.scalar.copy(m2_sb[:, :], m2[:, :])
                    m4_sb = tpool.tile([P, 512], f32, tag="t")
                    nc.scalar.copy(m4_sb[:, :], m4[:, :])
                    t11 = tpool.tile([P, 512], f32, tag="t")
                    nc.vector.tensor_add(t11[:, :], m1[:, :], m4_sb[:, :])
                    t22 = tpool.tile([P, 512], f32, tag="t")
                    nc.vector.tensor_sub(t22[:, :], m1[:, :], m2_sb[:, :])
                    t21 = tpool.tile([P, 512], f32, tag="t")
                    nc.vector.tensor_add(t21[:, :], m2_sb[:, :], m4_sb[:, :])

                    m5 = product(4)
                    m5_sb = tpool.tile([P, 512], f32, tag="t")
                    nc.scalar.copy(m5_sb[:, :], m5[:, :])
                    m3 = product(2)
                    # E12/E21 before the M7/M6 folds: their DVE chains (C21 is
                    # the shortest) start while M7/M6 matmuls still run, and the
                    # kernel tail isn't gated on e21 being the last psum
                    e12 = pspool.tile([P, 512], f32, tag="ps")
                    emit_dr(e12, x8_top, cR, start=True)
                    e21 = pspool.tile([P, 512], f32, tag="ps")
                    emit_dr(e21, x8_bot, cL, start=True)
                    m7 = product(6, fold=(x8_top, cL))   # + E11
                    m6 = product(5, fold=(x8_bot, cR))   # + E22

                    r_top = r * P
                    r_bot = MH + r * P

                    # C11 = t11 - M5 + M7'
                    u1 = tpool.tile([P, 512], f32, tag="t")
                    nc.vector.tensor_sub(u1[:, :], t11[:, :], m5_sb[:, :])
                    w1 = tpool.tile([P, 512], f32, tag="t")
                    nc.vector.tensor_add(w1[:, :], u1[:, :], m7[:, :])
                    o11 = opool.tile([P, 512], f32, tag="o")
                    nc.vector.scalar_tensor_tensor(
                        out=o11[:, :], in0=w1[:, :], scalar=1.0 / W_SCALE,
                        in1=bias_sb[:, cL : cL + 512],
                        op0=ALU.mult, op1=ALU.add,
                    )
                    nc.scalar.dma_start(
                        out=out[r_top : r_top + P, cL : cL + 512], in_=o11[:, :]
                    )
                    # C12 = M3 + M5 + E12
                    u3 = tpool.tile([P, 512], f32, tag="t")
                    nc.vector.tensor_add(u3[:, :], m3[:, :], m5_sb[:, :])
                    w3 = tpool.tile([P, 512], f32, tag="t")
                    nc.vector.tensor_add(w3[:, :], u3[:, :], e12[:, :])
                    o12 = opool.tile([P, 512], f32, tag="o")
                    nc.vector.scalar_tensor_tensor(
                        out=o12[:, :], in0=w3[:, :], scalar=1.0 / W_SCALE,
                        in1=bias_sb[:, cR : cR + 512],
                        op0=ALU.mult, op1=ALU.add,
                    )
                    nc.scalar.dma_start(
                        out=out[r_top : r_top + P, cR : cR + 512], in_=o12[:, :]
                    )
                    # C21 = t21 + E21
                    w4 = tpool.tile([P, 512], f32, tag="t")
                    nc.vector.tensor_add(w4[:, :], t21[:, :], e21[:, :])
                    o21 = opool.tile([P, 512], f32, tag="o")
                    nc.vector.scalar_tensor_tensor(
                        out=o21[:, :], in0=w4[:, :], scalar=1.0 / W_SCALE,
                        in1=bias_sb[:, cL : cL + 512],
                        op0=ALU.mult, op1=ALU.add,
                    )
                    nc.scalar.dma_start(
                        out=out[r_bot : r_bot + P, cL : cL + 512], in_=o21[:, :]
                    )
                    # C22 = t22 + M3 + M6'
                    u2 = tpool.tile([P, 512], f32, tag="t")
                    nc.vector.tensor_add(u2[:, :], t22[:, :], m3[:, :])
                    w2 = tpool.tile([P, 512], f32, tag="t")
                    nc.vector.tensor_add(w2[:, :], u2[:, :], m6[:, :])
                    o22 = opool.tile([P, 512], f32, tag="o")
                    nc.vector.scalar_tensor_tensor(
                        out=o22[:, :], in0=w2[:, :], scalar=1.0 / W_SCALE,
                        in1=bias_sb[:, cR : cR + 512],
                        op0=ALU.mult, op1=ALU.add,
                    )
                    nc.scalar.dma_start(
                        out=out[r_bot : r_bot + P, cR : cR + 512], in_=o22[:, :]
                    )
    nc.compile()
    return nc


def make_in_maps(input_, weight, bias):
    e4m3 = ml_dtypes.float8_e4m3
    X = np.asarray(input_, dtype=np.float32).reshape(M, H)
    X16 = X[:, :K16]
    A11 = X16[:MH, :KH]
    A12 = X16[:MH, KH:]
    A21 = X16[MH:, :KH]
    A22 = X16[MH:, KH:]
    acombos = [A11 + A22, A21 + A22, A11, A22, A11 + A12, A21 - A11, A12 - A22]
    # at[p, r, k, kt, m] = Acombo_p[r*P+m, kt*P+k]
    at = np.stack(
        [
            a.astype(np.float16).reshape(RT, P, KTH, P).transpose(0, 3, 2, 1)
            for a in acombos
        ]
    )
    at = np.ascontiguousarray(at)
    X8 = X[:, K16:].astype(e4m3)
    xt8 = np.ascontiguousarray(X8.reshape(2 * RT, P, A8, 2, P).transpose(0, 4, 2, 3, 1))
    b = np.asarray(bias, dtype=np.float32)
    Wall = np.asarray(weight, dtype=np.float32) * W_SCALE
    in_maps = []
    for c in range(N_CORES):
        Wc = Wall[c * FS : (c + 1) * FS]  # [FS, H]
        Bm = Wc[:, :K16].T  # [K16, FS]
        B11 = Bm[:KH, :FH]
        B12 = Bm[:KH, FH:]
        B21 = Bm[KH:, :FH]
        B22 = Bm[KH:, FH:]
        bcombos = [B11 + B22, B11, B12 - B22, B21 - B11, B22, B11 + B12, B21 + B22]
        # bt[p, k, h, kt, f] = Bcombo_p[kt*P+k, h*512+f]
        btc = np.stack(
            [
                bm.astype(np.float16).reshape(KTH, P, 2, 512).transpose(1, 2, 0, 3)
                for bm in bcombos
            ]
        )
        btc = np.ascontiguousarray(btc)
        W8c = Wc[:, K16:].astype(e4m3)  # [FS, K8]
        wt8c = np.ascontiguousarray(W8c.reshape(FS, A8, 2, P).transpose(3, 1, 2, 0))
        bc = np.ascontiguousarray(
            np.broadcast_to(b[c * FS : (c + 1) * FS][None, :], (P, FS))
        )
        in_maps.append({"at": at, "bt": btc, "xt8": xt8, "wt8": wt8c, "bias": bc})
    return in_maps


_NC_CACHE = {}


def run_spmd(input_, weight, bias, trace=False, **kw):
    from concourse.bass_utils import run_bass_kernel_spmd

    if "full" not in _NC_CACHE:
        _NC_CACHE["full"] = build_nc()
    nc = _NC_CACHE["full"]
    in_maps = make_in_maps(input_, weight, bias)
    res = run_bass_kernel_spmd(
        nc, in_maps, core_ids=list(range(N_CORES)), trace=trace, **kw
    )
    outs = [np.asarray(res.results[c]["out"]) for c in range(N_CORES)]
    full = np.concatenate(outs, axis=1).reshape(S, B, F)
    return full, res


def kernel(input_, weight, bias):
    out, _ = run_spmd(input_, weight, bias, trace=False)
    return out


# revision 7
# speedup vs baseline: 1.0249x; 1.0053x over previous
"""Strassen-1 (fp16) + flat fp8-DoubleRow hybrid column-parallel linear.

out = input_ @ weight.T + bias, F-sharded 8 ways; per-core C[8192,2048].

The contraction K=4096 splits into K16 fp16 planes + K8 = 256*A8 fp8 planes.
 - fp16 part: one level of Strassen over (M, K16, F): 7 products, each
   [4096, K16/2] @ [K16/2, 1024], host precomputes operand combos.
   C11 = M1+M4-M5+M7; C12 = M3+M5; C21 = M2+M4; C22 = M1-M2+M3+M6.
 - fp8 part: plain e4m3 DoubleRow GEMM over K8 planes (no Strassen: the
   recombination would amplify fp8 error ~2x). Folded into the same psum
   banks: E11 -> M7 (single-use in C11), E22 -> M6 (single-use in C22);
   E12 / E21 get their own banks.
PE time ~= (0.875*(K16/K) + 0.5*(K8/K)) * fp16-roofline ~= 0.734 -> ~1.31 ms.
All W scaled by 64 (e4m3 normal range); copyback divides by 64 + bias.
"""

import os
import sys

import numpy as np
import ml_dtypes

for _p in ("/opt/trn_rl_repo", "/root/.axon_site/_ro/trn_rl_repo"):
    if os.path.isdir(_p) and _p not in sys.path:
        sys.path.insert(0, _p)

P = 128
S, B, H, F = 4096, 2, 4096, 16384
N_CORES = 8
M = S * B
FS = F // N_CORES
W_SCALE = 64.0

A8 = 6  # fp8 256-plane blocks (alpha = A8/16)
K8 = 256 * A8
K16 = H - K8          # 2560
KH = K16 // 2         # 1280 (Strassen half-K)
KTH = KH // P         # 10
MH = M // 2           # 4096
RT = MH // P          # 32 row tiles per half
FH = FS // 2          # 1024 (abstract half-F)


def build_nc():
    from concourse import bacc
    import concourse.mybir as mybir
    import concourse.tile as tile

    f32 = mybir.dt.float32
    fp16 = mybir.dt.float16
    fp8 = mybir.dt.float8e4
    DR = mybir.MatmulPerfMode.DoubleRow
    ALU = mybir.AluOpType

    nc = bacc.Bacc(None, target_bir_lowering=False)
    # at[p, r, k, kt, m] = fp16(Acombo_p[r*P + m, kt*P + k])
    at = nc.declare_dram_parameter("at", [7, RT, P, KTH, P], fp16, isOutput=False)
    # bt[p, k, h, kt, f] = fp16(64 * Bcombo_p[kt*P + k, h*512 + f])
    bt = nc.declare_dram_parameter("bt", [7, P, 2, KTH, 512], fp16, isOutput=False)
    # xt8[R, k, j, i, m] = fp8(x[R*P + m, K16 + j*256 + i*128 + k])
    xt8 = nc.declare_dram_parameter("xt8", [2 * RT, P, A8, 2, P], fp8, isOutput=False)
    # wt8[k, j, i, f] = fp8(64 * w[f, K16 + j*256 + i*128 + k])
    wt8 = nc.declare_dram_parameter("wt8", [P, A8, 2, FS], fp8, isOutput=False)
    bias = nc.declare_dram_parameter("bias", [P, FS], f32, isOutput=False)
    out = nc.declare_dram_parameter("out", [M, FS], f32, isOutput=True)

    with tile.TileContext(nc) as tc:
        with (
            tc.tile_pool(name="bpool7", bufs=8) as bpool7,
            tc.tile_pool(name="w8pool", bufs=A8) as w8pool,
            tc.tile_pool(name="apool", bufs=16) as apool,
            tc.tile_pool(name="x8pool", bufs=6) as x8pool,
            tc.tile_pool(name="tpool", bufs=14) as tpool,
            tc.tile_pool(name="opool", bufs=8) as opool,
            tc.tile_pool(name="biaspool", bufs=1) as biaspool,
            tc.tile_pool(name="psum", bufs=8, space="PSUM") as pspool,
        ):
            # w8/bias and two of the seven h=0 B tiles ride the sync queue,
            # emitted inside (h=0, r=0) after that iteration's A/x8 loads, so
            # both DMA rings deliver iteration 0's working set in parallel
            # (one scalar ring alone can't keep up with 7 products' B demand)
            bias_sb = biaspool.tile([P, FS], f32)
            w8_kt = []
            for j in range(A8):
                wk8 = w8pool.tile([P, 2, FS], fp8, tag="w8kt")
                w8_kt.append(wk8)

            def emit_dr(ps, x8t, ocol0, start):
                # full-bank DR matmuls: rhs [128,2,512] streams fp8 pairs, out
                # free = 512 stays at the psum-bank limit; halves the DR MM count
                for j in range(A8):
                    nc.tensor.matmul(
                        ps[:, :],
                        lhsT=x8t[:, j, :, :],
                        rhs=w8_kt[j][:, :, ocol0 : ocol0 + 512],
                        start=(start and j == 0),
                        stop=(j == A8 - 1),
                        perf_mode=DR,
                    )

            for h in range(2):
                b_sb = []
                # defer b2/b5 to the sync queue inside r=0 of each half
                # (products M3/M6 need them latest); scalar delivers
                # b0,b1,b3,b4,b6 in need order
                deferred = (2, 5)
                scalar_order = [p for p in (0, 1, 3, 4, 6) if p not in deferred]
                btiles = {}
                for p in range(7):
                    btile = bpool7.tile([P, KTH, 512], fp16, tag="btile")
                    btiles[p] = btile
                    b_sb.append(btile)
                for p in scalar_order:
                    # kt-granular chunks: the first matmuls of this half wait on
                    # 250KB, not the full 8.75MB B reload (h-boundary stall)
                    for k0 in range(0, KTH, 2):
                        nc.scalar.dma_start(
                            out=btiles[p][:, k0 : k0 + 2, :],
                            in_=bt[p, :, h, k0 : k0 + 2, :],
                        )
                cL = h * 512          # C11 / C21 out-col base
                cR = FH + h * 512     # C12 / C22 out-col base
                for r in range(RT):
                    a_sb = []
                    for p in range(7):
                        atile = apool.tile([P, KTH, P], fp16, tag="atile")
                        nc.sync.dma_start(out=atile[:, :, :], in_=at[p, r, :, :, :])
                        a_sb.append(atile)
                    x8_top = x8pool.tile([P, A8, 2, P], fp8, tag="x8")
                    nc.sync.dma_start(out=x8_top[:, :, :, :], in_=xt8[r, :, :, :, :])
                    x8_bot = x8pool.tile([P, A8, 2, P], fp8, tag="x8")
                    nc.sync.dma_start(
                        out=x8_bot[:, :, :, :], in_=xt8[RT + r, :, :, :, :]
                    )
                    if r == 0:
                        # startup extras on the sync ring in need order: w8
                        # first (E12 folds hit it ~13us in), then b2 (M3,
                        # ~11us), b5 (M6, last product), bias (~20us)
                        if h == 0:
                            for j in range(A8):
                                nc.sync.dma_start(
                                    out=w8_kt[j][:, :, :], in_=wt8[:, j, :, :]
                                )
                        for p in deferred:
                            for k0 in range(0, KTH, 2):
                                nc.sync.dma_start(
                                    out=btiles[p][:, k0 : k0 + 2, :],
                                    in_=bt[p, :, h, k0 : k0 + 2, :],
                                )
                        if h == 0:
                            # bias rides the scalar ring (sync is the startup
                            # long pole and delays iteration 1-2 A tiles)
                            nc.scalar.dma_start(out=bias_sb[:, :], in_=bias[:, :])

                    def product(p_idx, fold=None):
                        ps = pspool.tile([P, 512], f32, tag="ps")
                        for kt in range(KTH):
                            nc.tensor.matmul(
                                ps[:, :],
                                lhsT=a_sb[p_idx][:, kt, :],
                                rhs=b_sb[p_idx][:, kt, :],
                                start=(kt == 0),
                                stop=(kt == KTH - 1 and fold is None),
                            )
                        if fold is not None:
                            emit_dr(ps, fold[0], fold[1], start=False)
                        return ps

                    # early-freed products first (bank reuse across the 9 tiles).
                    # DVE reads at most one PSUM operand per op, so m2/m4/m5 go
                    # through the (otherwise idle) scalar engine to SBUF first.
                    m1 = product(0)
                    m2 = product(1)
                    m4 = product(3)
                    m2_sb = tpool.tile([P, 512], f32, tag="t")
                    nc.scalar.copy(m2_sb[:, :], m2[:, :])
                    m4_sb = tpool.tile([P, 512], f32, tag="t")
                    nc.scalar.copy(m4_sb[:, :], m4[:, :])
                    t11 = tpool.tile([P, 512], f32, tag="t")
                    nc.vector.tensor_add(t11[:, :], m1[:, :], m4_sb[:, :])
                    t22 = tpool.tile([P, 512], f32, tag="t")
                    nc.vector.tensor_sub(t22[:, :], m1[:, :], m2_sb[:, :])
                    t21 = tpool.tile([P, 512], f32, tag="t")
                    nc.vector.tensor_add(t21[:, :], m2_sb[:, :], m4_sb[:, :])

                    m5 = product(4)
                    m5_sb = tpool.tile([P, 512], f32, tag="t")
                    nc.scalar.copy(m5_sb[:, :], m5[:, :])
                    m3 = product(2)
                    # E12/E21 before the M7/M6 folds: their DVE chains (C21 is
                    # the shortest) start while M7/M6 matmuls still run, and the
                    # kernel tail isn't gated on e21 being the last psum
                    e12 = pspool.tile([P, 512], f32, tag="ps")
                    emit_dr(e12, x8_top, cR, start=True)
                    e21 = pspool.tile([P, 512], f32, tag="ps")
                    emit_dr(e21, x8_bot, cL, start=True)
                    m7 = product(6, fold=(x8_top, cL))   # + E11
                    m6 = product(5, fold=(x8_bot, cR))   # + E22

                    r_top = r * P
                    r_bot = MH + r * P

                    # C11 = t11 - M5 + M7'
                    u1 = tpool.tile([P, 512], f32, tag="t")
                    nc.vector.tensor_sub(u1[:, :], t11[:, :], m5_sb[:, :])
                    w1 = tpool.tile([P, 512], f32, tag="t")
                    nc.vector.tensor_add(w1[:, :], u1[:, :], m7[:, :])
                    o11 = opool.tile([P, 512], f32, tag="o")
                    nc.vector.scalar_tensor_tensor(
                        out=o11[:, :], in0=w1[:, :], scalar=1.0 / W_SCALE,
                        in1=bias_sb[:, cL : cL + 512],
                        op0=ALU.mult, op1=ALU.add,
                    )
                    nc.scalar.dma_start(
                        out=out[r_top : r_top + P, cL : cL + 512], in_=o11[:, :]
                    )
                    # C12 = M3 + M5 + E12
                    u3 = tpool.tile([P, 512], f32, tag="t")
                    nc.vector.tensor_add(u3[:, :], m3[:, :], m5_sb[:, :])
                    w3 = tpool.tile([P, 512], f32, tag="t")
                    nc.vector.tensor_add(w3[:, :], u3[:, :], e12[:, :])
                    o12 = opool.tile([P, 512], f32, tag="o")
                    nc.vector.scalar_tensor_tensor(
                        out=o12[:, :], in0=w3[:, :], scalar=1.0 / W_SCALE,
                        in1=bias_sb[:, cR : cR + 512],
                        op0=ALU.mult, op1=ALU.add,
                    )
                    nc.scalar.dma_start(
                        out=out[r_top : r_top + P, cR : cR + 512], in_=o12[:, :]
                    )
                    # C21 = t21 + E21
                    w4 = tpool.tile([P, 512], f32, tag="t")
                    nc.vector.tensor_add(w4[:, :], t21[:, :], e21[:, :])
                    o21 = opool.tile([P, 512], f32, tag="o")
                    nc.vector.scalar_tensor_tensor(
                        out=o21[:, :], in0=w4[:, :], scalar=1.0 / W_SCALE,
                        in1=bias_sb[:, cL : cL + 512],
                        op0=ALU.mult, op1=ALU.add,
                    )
                    nc.scalar.dma_start(
                        out=out[r_bot : r_bot + P, cL : cL + 512], in_=o21[:, :]
                    )
                    # C22 = t22 + M3 + M6'
                    u2 = tpool.tile([P, 512], f32, tag="t")
                    nc.vector.tensor_add(u2[:, :], t22[:, :], m3[:, :])
                    w2 = tpool.tile([P, 512], f32, tag="t")
                    nc.vector.tensor_add(w2[:, :], u2[:, :], m6[:, :])
                    o22 = opool.tile([P, 512], f32, tag="o")
                    nc.vector.scalar_tensor_tensor(
                        out=o22[:, :], in0=w2[:, :], scalar=1.0 / W_SCALE,
                        in1=bias_sb[:, cR : cR + 512],
                        op0=ALU.mult, op1=ALU.add,
                    )
                    nc.scalar.dma_start(
                        out=out[r_bot : r_bot + P, cR : cR + 512], in_=o22[:, :]
                    )
    nc.compile()
    return nc


def make_in_maps(input_, weight, bias):
    e4m3 = ml_dtypes.float8_e4m3
    X = np.asarray(input_, dtype=np.float32).reshape(M, H)
    X16 = X[:, :K16]
    A11 = X16[:MH, :KH]
    A12 = X16[:MH, KH:]
    A21 = X16[MH:, :KH]
    A22 = X16[MH:, KH:]
    acombos = [A11 + A22, A21 + A22, A11, A22, A11 + A12, A21 - A11, A12 - A22]
    # at[p, r, k, kt, m] = Acombo_p[r*P+m, kt*P+k]
    at = np.stack(
        [
            a.astype(np.float16).reshape(RT, P, KTH, P).transpose(0, 3, 2, 1)
            for a in acombos
        ]
    )
    at = np.ascontiguousarray(at)
    X8 = X[:, K16:].astype(e4m3)
    xt8 = np.ascontiguousarray(X8.reshape(2 * RT, P, A8, 2, P).transpose(0, 4, 2, 3, 1))
    b = np.asarray(bias, dtype=np.float32)
    Wall = np.asarray(weight, dtype=np.float32) * W_SCALE
    in_maps = []
    for c in range(N_CORES):
        Wc = Wall[c * FS : (c + 1) * FS]  # [FS, H]
        Bm = Wc[:, :K16].T  # [K16, FS]
        B11 = Bm[:KH, :FH]
        B12 = Bm[:KH, FH:]
        B21 = Bm[KH:, :FH]
        B22 = Bm[KH:, FH:]
        bcombos = [B11 + B22, B11, B12 - B22, B21 - B11, B22, B11 + B12, B21 + B22]
        # bt[p, k, h, kt, f] = Bcombo_p[kt*P+k, h*512+f]
        btc = np.stack(
            [
                bm.astype(np.float16).reshape(KTH, P, 2, 512).transpose(1, 2, 0, 3)
                for bm in bcombos
            ]
        )
        btc = np.ascontiguousarray(btc)
        W8c = Wc[:, K16:].astype(e4m3)  # [FS, K8]
        wt8c = np.ascontiguousarray(W8c.reshape(FS, A8, 2, P).transpose(3, 1, 2, 0))
        bc = np.ascontiguousarray(
            np.broadcast_to(b[c * FS : (c + 1) * FS][None, :], (P, FS))
        )
        in_maps.append({"at": at, "bt": btc, "xt8": xt8, "wt8": wt8c, "bias": bc})
    return in_maps


_NC_CACHE = {}


def run_spmd(input_, weight, bias, trace=False, **kw):
    from concourse.bass_utils import run_bass_kernel_spmd

    if "full" not in _NC_CACHE:
        _NC_CACHE["full"] = build_nc()
    nc = _NC_CACHE["full"]
    in_maps = make_in_maps(input_, weight, bias)
    res = run_bass_kernel_spmd(
        nc, in_maps, core_ids=list(range(N_CORES)), trace=trace, **kw
    )
    outs = [np.asarray(res.results[c]["out"]) for c in range(N_CORES)]
    full = np.concatenate(outs, axis=1).reshape(S, B, F)
    return full, res


def kernel(input_, weight, bias):
    out, _ = run_spmd(input_, weight, bias, trace=False)
    return out
